# revision 17
# baseline (speedup 1.0000x reference)
"""Single-head causal attention (B=8, T=2048, D=512, H=64) on 8 TRN2 cores.

Data-parallel: one batch element per NeuronCore. Each core computes
attention in the S^T layout (keys on partitions, queries on the free axis):

  qT/kT/vT [64, T] = W.T @ x.T        (fp16 matmuls, 4 c-tile accumulation)
  v1       [128, 16, 65]              (DMA-xbar transpose of vT + ones col)
  S^T[j,i] = kT_jblock.T @ qT          (strips of causal width)
  P^T      = exp(S^T / 8)              (ScalarE; no max-subtraction: scores
                                        are bounded by ~|q||k|sqrt(H)/8 << 88)
  out^T[h,i], l[i] = [v|1]_jb.T @ P^T  (accumulated over j-blocks in PSUM;
                                        row 64 is the softmax denominator)

The host divides by the denominator row and transposes back to [T, 64].

Schedule: the exp stream on ACT (~19us busy at ~1.09ns/col) and the PE
stream (~21.5us of moving columns at 2.4GHz warm) are co-critical; input
DMA is the head constraint (each HWDGE queue sustains only ~100GB/s, so
x chunk 1 - which every pass-0 S strip needs - is pulled over the Pool
engine's software-DGE queue as a third DMA channel while the two HWDGE
queues carry x0+consts and x2/x3).  The kernel keeps ACT saturated with
a 2-deep software pipeline per j-block - emitted as exp(jb); S(jb+2);
PV(jb) - where S(jb+2) reuses S(jb)'s PSUM tile, so its write-after-read
wait gates it right behind the activation; pass 1's first two strips are
computed in pass 0's last weave slots so the exp stream crosses the pass
boundary without a gap, and eight p-tile buffers let the exps run well
ahead of the PV accumulation.  Causal-mask multiplies run off the ACT/PE
critical engines: on the Pool engine, alternating with the DVE through
pass 1's tail where eight diagonal masks arrive faster than one Pool
queue drains them.  out_sb has 6 buffers, the final 256-column piece is
copied on the post-exp-idle ACT engine and row-split across both DMA
queues, keeping the post-compute drain short.  A dense warm-up burst
holds the PE HAM clock-gate open until data lands.
"""

import sys

sys.path.insert(0, "/opt/trn_rl_repo")

import numpy as np

import concourse.bass as bass
import concourse.mybir as mybir
import concourse.tile as tile

B, T, D, H = 8, 2048, 512, 64
N_CORES = 8
HALF = T // 2  # i-axis pass width
NC_TILES = D // 128  # 4 c-tiles
NCH = T // 512  # 4 t-chunks

f32 = mybir.dt.float32
f16 = mybir.dt.float16
bf16 = mybir.dt.bfloat16

_cache = {}

CW = 512 + 256 + 128  # wqk | wv | mask
XCOLS = CW + NCH * NC_TILES * 512  # consts followed by x chunks


def _legalize_waits(nc, max_waits=1):
    counter = 0
    for bb in nc.main_func.blocks:
        if not any(
            ins.sync_info is not None and len(ins.sync_info.on_wait) > max_waits
            for ins in bb.instructions
        ):
            continue
        new_list = []
        for ins in bb.instructions:
            si = ins.sync_info
            if si is not None and len(si.on_wait) > max_waits:
                waits = list(si.on_wait)
                hoist, keep = waits[:-max_waits], waits[-max_waits:]
                for w in hoist:
                    counter += 1
                    new_list.append(
                        mybir.InstNoOp(
                            name=f"I-waitfix-{counter}",
                            engine=ins.engine,
                            sync_info=mybir.SyncInfo(on_wait=[w], on_update=[]),
                            bass_nofuse=True,
                        )
                    )
                ins.sync_info = mybir.SyncInfo(
                    on_wait=keep, on_update=list(si.on_update)
                )
            new_list.append(ins)
        bb.instructions = new_list
    return counter


def _chunks(lo, hi, step, align):
    out = []
    cur = lo
    while cur < hi:
        nxt = min(hi, align + ((cur - align) // step + 1) * step)
        out.append((cur, nxt))
        cur = nxt
    return out


def _build():
    nc = bass.Bass()

    xfull_d = nc.declare_dram_parameter("xfull", [128, XCOLS], f16, isOutput=False)
    out_d = nc.declare_dram_parameter("out", [H + 1, T], f32, isOutput=True)

    with tile.TileContext(nc) as tc:
        with (
            tc.tile_pool(name="xt", bufs=1) as xpool,
            tc.tile_pool(name="qkv", bufs=1) as qkvpool,
            tc.tile_pool(name="p", bufs=2) as ppool,
            tc.tile_pool(name="o", bufs=2) as opool,
            tc.tile_pool(name="ps_proj", bufs=2, space="PSUM") as ps_proj,
            tc.tile_pool(name="ps_s", bufs=2, space="PSUM") as ps_s,
            tc.tile_pool(name="ps_pv", bufs=1, space="PSUM") as ps_pv,
        ):
            xall = xpool.tile([128, XCOLS], f16)
            wqk = [xall[:, 128 * c : 128 * (c + 1)] for c in range(NC_TILES)]
            wv = [xall[:, 512 + 64 * c : 512 + 64 * (c + 1)] for c in range(NC_TILES)]
            mask16 = xall[:, 768:896]

            def xc(k, c):  # x chunk k, c-tile c: [128, 512]
                base = CW + 2048 * k + 512 * c
                return xall[:, base : base + 512]

            warm_bf = xpool.tile([128, 512], bf16)
            nc.gpsimd.memset(warm_bf[:], 1.0)

            def dpiece(eng, lo, hi):
                eng.dma_start(xall[:, lo:hi], xfull_d[:, lo:hi])

            x0 = CW
            dpiece(nc.sync, x0, x0 + 2048)  # x chunk 0
            dpiece(nc.scalar, 0, CW)  # consts
            dpiece(nc.gpsimd, x0 + 2048, x0 + 4096)  # x chunk 1 (SWDGE)
            dpiece(nc.sync, x0 + 4096, x0 + 6144)  # x chunk 2
            dpiece(nc.scalar, x0 + 6144, x0 + 8192)  # x chunk 3

            exp_warm = xpool.tile([1, 2], f32)
            nc.scalar.activation(
                exp_warm[:], warm_bf[0:1, 0:2], mybir.ActivationFunctionType.Exp
            )

            qT = qkvpool.tile([H, T], f16)
            kT = qkvpool.tile([H, T], f16)
            vT = qkvpool.tile([H, T], f16)
            v1 = qkvpool.tile([128, T // 128, H + 1], f16)
            nc.gpsimd.memset(v1[:, :, H : H + 1], 1.0)

            warm_ps = ps_proj.tile([128, 512], f32, tag="work", name="warm_ps")
            for _ in range(10):
                nc.tensor.matmul(
                    warm_ps[:], warm_bf[:, 0:128], warm_bf[:], start=True, stop=True
                )

            def proj_qk_mm(k):
                ps = ps_proj.tile([128, 512], f32, tag="work", name="qk_ps")
                for c in range(NC_TILES):
                    nc.tensor.matmul(
                        ps[:], wqk[c], xc(k, c),
                        start=(c == 0), stop=(c == NC_TILES - 1),
                    )
                return ps

            def proj_qk_copy(k, ps):
                if k == 0:
                    nc.scalar.copy(qT[:, 0:512], ps[0:H, :])
                else:
                    nc.vector.tensor_copy(qT[:, 512 * k : 512 * (k + 1)], ps[0:H, :])
                nc.vector.tensor_copy(kT[:, 512 * k : 512 * (k + 1)], ps[H : 2 * H, :])

            def proj_v_mm(k):
                ps = ps_proj.tile([128, 512], f32, tag="work", name="v_ps")
                for c in range(NC_TILES):
                    nc.tensor.matmul(
                        ps[0:H, :], wv[c], xc(k, c),
                        start=(c == 0), stop=(c == NC_TILES - 1),
                    )
                return ps

            def proj_v_copy(k, ps):
                nc.vector.tensor_copy(vT[:, 512 * k : 512 * (k + 1)], ps[0:H, :])
                v1s = opool.tile([128, 4, H], f16, tag="v1s", name="v1s", bufs=2)
                nc.sync.dma_start_transpose(v1s[:], vT[:, 512 * k : 512 * (k + 1)])
                nc.gpsimd.tensor_copy(v1[:, 4 * k : 4 * (k + 1), 0:H], v1s[:])

            def proj_qk(k):
                proj_qk_copy(k, proj_qk_mm(k))

            def proj_v(k):
                proj_v_copy(k, proj_v_mm(k))

            def attn_S(t0, jb):
                i_start = max(t0, 128 * jb)
                W = t0 + HALF - i_start
                s_ps = ps_s.tile([128, 1024], f32, tag="s", name="s_ps")
                for ls, le in _chunks(0, W, 512, 0):
                    nc.tensor.matmul(
                        s_ps[:, ls:le],
                        kT[:, 128 * jb : 128 * (jb + 1)],
                        qT[:, i_start + ls : i_start + le],
                        start=True,
                        stop=True,
                    )
                return s_ps

            def attn_exp(t0, jb, s_ps, split=False):
                i_start = max(t0, 128 * jb)
                W = t0 + HALF - i_start
                p_sb = ppool.tile([128, 1024], f16, tag="p", name="p_sb", bufs=8)
                bounds = [0, 512, W] if (split and W > 512) else [0, W]
                for lo, hi in zip(bounds[:-1], bounds[1:]):
                    nc.scalar.activation(
                        p_sb[:, lo:hi],
                        s_ps[:, lo:hi],
                        mybir.ActivationFunctionType.Exp,
                        scale=1.0 / 8.0,
                    )
                if 128 * jb >= t0:
                    # pass 1's tail has 8 diagonal masks in quick succession;
                    # one Pool-engine queue (430ns/mask) falls behind the
                    # shrinking exps, so alternate with the mostly-idle DVE
                    eng = nc.vector if (t0 == HALF and jb % 2 == 1) else nc.gpsimd
                    eng.tensor_mul(p_sb[:, 0:128], p_sb[:, 0:128], mask16)
                return p_sb

            def attn_pv(t0, n_jb, pv_ps, jb, p_sb):
                i_start = max(t0, 128 * jb)
                pieces = _chunks(i_start, t0 + HALF, 512, 0)
                if 128 * jb >= t0 and len(pieces) > 1:
                    pieces = pieces[1:] + pieces[:1]
                for gs, ge in pieces:
                    ic_last_jb = min(n_jb - 1, (ge - 1) // 128)
                    nc.tensor.matmul(
                        pv_ps[:, gs - t0 : ge - t0],
                        v1[:, jb, :],
                        p_sb[:, gs - i_start : ge - i_start],
                        start=(jb == 0),
                        stop=(jb == ic_last_jb),
                    )

            def out_piece(pv_ps, t0, lo, hi, split_rows=False):
                out_sb = opool.tile([H + 1, 512], f32, tag="o", name="out_sb", bufs=6)
                if split_rows:
                    # final piece: ACT is idle after the last exp, DVE isn't
                    nc.scalar.copy(out_sb[:, 0 : hi - lo], pv_ps[:, lo:hi])
                else:
                    nc.vector.tensor_copy(out_sb[:, 0 : hi - lo], pv_ps[:, lo:hi])
                if split_rows:
                    nc.sync.dma_start(
                        out_d[0:33, t0 + lo : t0 + hi], out_sb[0:33, 0 : hi - lo]
                    )
                    nc.scalar.dma_start(
                        out_d[33:, t0 + lo : t0 + hi], out_sb[33:, 0 : hi - lo]
                    )
                else:
                    nc.sync.dma_start(
                        out_d[:, t0 + lo : t0 + hi], out_sb[:, 0 : hi - lo]
                    )

            ps_qk0 = proj_qk_mm(0)
            proj_qk_copy(0, ps_qk0)
            ps_v0 = proj_v_mm(0)
            s00 = ps_s.tile([128, 1024], f32, tag="s", name="s_ps")
            nc.tensor.matmul(
                s00[:, 0:512], kT[:, 0:128], qT[:, 0:512], start=True, stop=True
            )
            proj_v_copy(0, ps_v0)
            ps_qk1 = proj_qk_mm(1)
            proj_qk_copy(1, ps_qk1)
            nc.tensor.matmul(
                s00[:, 512:1024], kT[:, 0:128], qT[:, 512:1024],
                start=True, stop=True,
            )
            proj_v(1)

            def attn_pass(t0, n_jb, weave, outs, split_jb, s0=None, s1=None):
                pv_ps = ps_pv.tile([H + 1, HALF], f32, tag="pv", name="pv_ps")
                s = {
                    0: s0 if s0 is not None else attn_S(t0, 0),
                    1: s1 if s1 is not None else attn_S(t0, 1),
                }
                for jb in range(n_jb):
                    p_sb = attn_exp(t0, jb, s.pop(jb), split=(jb in split_jb))
                    if jb + 2 < n_jb:
                        s[jb + 2] = attn_S(t0, jb + 2)
                    attn_pv(t0, n_jb, pv_ps, jb, p_sb)
                    if jb in weave:
                        weave[jb]()
                    if jb in outs:
                        out_piece(pv_ps, t0, *outs[jb])
                out_piece(
                    pv_ps, t0, HALF - 512 + 256, HALF,
                    split_rows=(t0 == HALF),
                )
                return pv_ps

            st = {}
            attn_pass(
                0, 8,
                weave={
                    3: lambda: proj_qk(2),
                    5: lambda: proj_qk(3),
                    # pass 1's first two strips only need q2/q3 + kT[0:256];
                    # computing them here removes the pass-boundary exp gap
                    6: lambda: st.__setitem__("s1_0", attn_S(HALF, 0)),
                    7: lambda: st.__setitem__("s1_1", attn_S(HALF, 1)),
                },
                outs={3: (0, 512), 6: (512, 768)},
                split_jb=(0, 1),
                s0=s00,
            )

            attn_pass(
                HALF, 16,
                weave={1: lambda: proj_v(2), 5: lambda: proj_v(3)},
                outs={11: (0, 512), 13: (512, 768)},
                split_jb=(),
                s0=st["s1_0"],
                s1=st["s1_1"],
            )

    _legalize_waits(nc)
    return nc


def build_in_maps(x, Wq, Wk, Wv):
    x = np.ascontiguousarray(np.asarray(x), dtype=np.float32)
    wqk_np = np.ascontiguousarray(
        np.concatenate([np.asarray(Wq), np.asarray(Wk)], axis=1), dtype=np.float32
    )
    wv_np = np.ascontiguousarray(np.asarray(Wv), dtype=np.float32)

    def ctile_pack(a, w):
        return a.reshape(4, 128, w).transpose(1, 0, 2).reshape(128, 4 * w)

    mask_np = np.triu(np.ones((128, 128), dtype=np.float16))
    consts_np = np.concatenate(
        [
            ctile_pack(wqk_np.astype(np.float16), 128),
            ctile_pack(wv_np.astype(np.float16), 64),
            mask_np,
        ],
        axis=1,
    )

    maps = []
    for b in range(N_CORES):
        xb = x[b].astype(np.float16)
        xpart = xb.reshape(NCH, 512, NC_TILES, 128).transpose(3, 0, 2, 1)
        xfull = np.ascontiguousarray(
            np.concatenate([consts_np, xpart.reshape(128, -1)], axis=1)
        )
        maps.append({"xfull": xfull})
    return maps


def kernel(x, Wq, Wk, Wv):
    from concourse.bass_utils import run_bass_kernel_spmd

    if "nc" not in _cache:
        _cache["nc"] = _build()
    nc = _cache["nc"]

    in_maps = build_in_maps(x, Wq, Wk, Wv)
    res = run_bass_kernel_spmd(nc, in_maps, list(range(N_CORES))).results

    out = np.empty((B, T, H), dtype=np.float32)
    for b in range(N_CORES):
        strip = res[b]["out"]  # [H+1, T]
        out[b] = (strip[:H, :] / strip[H : H + 1, :]).T
    return out
